# revision 6
# baseline (speedup 1.0000x reference)
# Cross-attention kernel for Trainium2, 8 NeuronCores.
#
# Sharding: data-parallel over (batch, query-half): core = 2*b + half handles
# batch b, queries [half*1024, (half+1)*1024). No collectives.
#
# On-device layout is feature-major: activations live as [feature, token] in
# fp16. Both layernorms fold into the projections via the postscale form
#   LN(x) @ W'.T = (x @ W' + [-S; bq] x [m; std]) * rstd,
# so the PSUM->SBUF evacuation copy becomes the rstd multiply. The finale
# keeps the prescale form so Gelu reads its PSUM accumulator directly.
#
# v15 structure: the attention unit order is c-major (query-chunk outer
# across head-pairs) so the c=0 finale streams inside the exp window; V and
# the later K/Q projection chunks stream through the window as PE tasks under
# the ACT exp stream. Input DMAs ride four independent engine queues (sync:
# K, vector: V, scalar: weights+Q, gpsimd: small internal rows) so the
# critical K chunk + weights land in ~2us instead of queueing behind 5MB.
# All LN stats run in the prefix so the ACT engine never reloads the Exp
# table mid-stream; the two finale blocks each cost one Sqrt+Gelu table
# excursion. Output is written fp16 (absmax ~5, quantization ~3e-4 abs).
import os
import sys
import tempfile

os.environ["NEURON_COMPILE_CACHE_URL"] = tempfile.mkdtemp(prefix="neff_cache_")
os.environ["AXON_CASSETTE_SALT"] = f"ca-{os.getpid()}-{os.urandom(4).hex()}"

for _p in ("/opt/trn_rl_repo",):
    if os.path.isdir(_p) and _p not in sys.path:
        sys.path.insert(0, _p)

import numpy as np
from contextlib import ExitStack

import concourse.bass as bass
import concourse.tile as tile
from concourse import bacc, mybir
from concourse.bass_utils import run_bass_kernel_spmd

F32 = mybir.dt.float32
F32R = mybir.dt.float32r
F16 = mybir.dt.float16
AF = mybir.ActivationFunctionType
ALU = mybir.AluOpType

B, NQ, NK, D = 4, 2048, 2048, 512
H, DH = 8, 64
NQS = NQ // 2  # queries per core
TEMP = float(np.sqrt(512.0))
LN_EPS = 1e-5
N_CORES = 8

_CACHE = {}
SALT = "v15"
SALT_N = 15


def _build_program():
    nc = bacc.Bacc("TRN2", target_bir_lowering=False, debug=False)

    def din(name, shape, dt=F32):
        return nc.dram_tensor(f"{name}_{SALT}", shape, dt,
                              kind="ExternalInput").ap()

    qt_d = din("qt", [128, 4, NQS], F16)
    kt_d = din("kt", [128, 4, NK], F16)
    vt_d = din("vt", [128, 4, NK], F16)
    wq_d = din("wq", [128, 4, D], F16)
    wk_d = din("wk", [128, 4, D], F16)
    wv_d = din("wv", [128, 4, D], F16)
    wo_d = din("wo", [128, 4, D], F32R)
    aq_d = din("aq", [2, D], F32R)
    ak_d = din("ak", [2, D], F32R)
    ao_d = din("ao", [2, D], F32R)
    gb_d = din("gb", [128, 2, 4], F32)
    grow_d = din("grow", [1, D], F32R)
    salt_d = din("salt", [1, 8 + SALT_N], F32)
    out_d = nc.dram_tensor(f"out_{SALT}", [128, 4, NQS], F16,
                           kind="ExternalOutput").ap()

    with tile.TileContext(nc) as tc, ExitStack() as top:
        persist = top.enter_context(tc.tile_pool(name="persist", bufs=1))
        qTs = persist.tile([128, 4, NQS], F16)        # projected q
        kTs = persist.tile([128, 4, NK], F16)         # projected k
        vaug = persist.tile([128, 16, 8, 65], F16)    # v natural + ones col
        oTs = persist.tile([128, 4, NQS], F32R)       # attention out
        qtin = persist.tile([128, 4, NQS], F16)
        ktin = persist.tile([128, 4, NK], F16)
        vtin = persist.tile([128, 4, NK], F16)
        wq_sb = persist.tile([128, 4, D], F16)
        wk_sb = persist.tile([128, 4, D], F16)
        wv_sb = persist.tile([128, 4, D], F16)
        wo_sb = persist.tile([128, 4, D], F32R)
        aq_sb = persist.tile([2, D], F32R)
        ak_sb = persist.tile([2, D], F32R)
        ao_sb = persist.tile([2, D], F32R)
        gb_sb = persist.tile([128, 2, 4], F32)
        grow_sb = persist.tile([1, D], F32R)
        auxq = persist.tile([2, NQS], F32R)           # [m; std] rows for Q
        auxk = persist.tile([2, NK], F32R)
        auxo = persist.tile([2, NQS], F32R)           # [m*r; 1] rows, finale
        rstdQ = persist.tile([128, 2, 512], F32)      # replicated 1/std per
        rstdK = persist.tile([128, 4, 512], F32)      # 512-token chunk
        onesh = persist.tile([128, 128], F16)
        onesr = persist.tile([128, 128], F32R)
        eps_t = persist.tile([128, 1], F32)

        # Three independent DMA queues (only SP/ACT/gpsimd may trigger):
        # sync carries the bulk (K, V, finale weights); scalar triggers ONLY
        # the critical small loads then frees the ACT engine for the stats
        # chain (each 5th+ trigger on a queue waits on semaphore recycling,
        # so a long trigger list pins its engine for the whole load);
        # gpsimd stays near-empty for latency-critical rows issued
        # mid-stream.
        nc.scalar.dma_start(out=eps_t,
                            in_=salt_d[0:1, 0:1].to_broadcast([128, 1]))
        nc.scalar.dma_start(out=wk_sb, in_=wk_d)
        nc.scalar.dma_start(out=wq_sb, in_=wq_d)
        for n2 in range(4):
            ns = slice(512 * n2, 512 * n2 + 512)
            nc.sync.dma_start(out=ktin[:, :, ns], in_=kt_d[:, :, ns])
        for n2 in range(2):
            ns = slice(512 * n2, 512 * n2 + 512)
            nc.scalar.dma_start(out=qtin[:, :, ns], in_=qt_d[:, :, ns])
        nc.gpsimd.dma_start(out=aq_sb, in_=aq_d)
        nc.gpsimd.dma_start(out=ak_sb, in_=ak_d)
        nc.scalar.dma_start(out=wv_sb, in_=wv_d)
        for n2 in range(4):
            ns = slice(512 * n2, 512 * n2 + 512)
            nc.sync.dma_start(out=vtin[:, :, ns], in_=vt_d[:, :, ns])
        nc.sync.dma_start(out=gb_sb, in_=gb_d)
        nc.sync.dma_start(out=grow_sb, in_=grow_d)
        nc.sync.dma_start(out=ao_sb, in_=ao_d)
        nc.sync.dma_start(out=wo_sb, in_=wo_d)
        nc.vector.memset(onesh, 1.0)
        # memset cannot target f32r; write through an f32 view (same bytes)
        nc.vector.memset(onesr.bitcast(F32), 1.0)
        nc.vector.memset(vaug[:, :, :, 64], 1.0)
        # row 0 is overwritten with m*r in the finale; row 1 stays all-ones
        nc.vector.memset(auxo.bitcast(F32), 1.0)

        pmm = top.enter_context(tc.tile_pool(name="pmm", bufs=1, space="PSUM"))
        work = top.enter_context(tc.tile_pool(name="work", bufs=1))
        # Prefix-only stats accumulators, released before the attention
        # pools open.
        pre_ctx = tc.tile_pool(name="pre", bufs=1, space="PSUM")
        pre = pre_ctx.__enter__()

        def ln_stats(xin, n2, aux, rstd_store, pool):
            """Column LN stats of token chunk n2 of xin [128, 4, *]; fills
            aux rows [m; std] and rstd_store[:, n2, :] (replicated)."""
            ns = slice(512 * n2, 512 * n2 + 512)
            ps_sum = pool.tile([128, 512], F32, name="st", bufs=4)
            for kc in range(4):
                nc.tensor.matmul(ps_sum, onesh, xin[:, kc, ns],
                                 start=(kc == 0), stop=(kc == 3))
            ps_ssq = pool.tile([128, 512], F32, name="st", bufs=4)
            for kc in range(4):
                sq = work.tile([128, 512], F16, name="sq", bufs=3)
                with nc.allow_low_precision("squares in fp16"):
                    nc.vector.tensor_mul(sq, xin[:, kc, ns], xin[:, kc, ns])
                nc.tensor.matmul(ps_ssq, onesh, sq,
                                 start=(kc == 0), stop=(kc == 3))
            m_b = work.tile([128, 512], F32, name="w32", bufs=6)
            nc.scalar.mul(m_b, ps_sum, 1.0 / 512.0)
            t2 = work.tile([128, 512], F32, name="w32", bufs=6)
            nc.vector.tensor_mul(t2, m_b, ps_sum)
            dv = work.tile([128, 512], F32, name="w32", bufs=6)
            nc.vector.tensor_sub(dv, ps_ssq, t2)
            std_b = work.tile([128, 512], F32, name="w32", bufs=6)
            nc.scalar.activation(std_b, dv, AF.Sqrt, bias=eps_t,
                                 scale=1.0 / 512.0)
            nc.vector.reciprocal_approx_fast(rstd_store[:, n2, :], std_b)
            # engine ops cannot move data across partitions; DMA the stat
            # rows into the K=2 aux operand instead (gpsimd queue: empty,
            # 25ns trigger).
            nc.gpsimd.dma_start(out=aux[0:1, ns].bitcast(F32), in_=m_b[0:1, :])
            nc.gpsimd.dma_start(out=aux[1:2, ns].bitcast(F32),
                                in_=std_b[0:1, :])

        def proj_chunk(dst, xin, w_sb, a_sb, aux, rstd_store, jc, n2):
            """dst[:, jc, ns] = (sum_kc w'[:,kc,js].T @ x[:,kc,ns] + aux)*r."""
            ns = slice(512 * n2, 512 * n2 + 512)
            js = slice(128 * jc, 128 * jc + 128)
            pg = pmm.tile([128, 512], F32, name="pmm", bufs=2)
            for kc in range(4):
                nc.tensor.matmul(pg, w_sb[:, kc, js], xin[:, kc, ns],
                                 start=(kc == 0), stop=False)
            nc.tensor.matmul(pg, a_sb[:, js], aux[:, ns],
                             start=False, stop=True)
            with nc.allow_low_precision("fp16 activations"):
                nc.vector.tensor_mul(dst[:, jc, ns], pg, rstd_store[:, n2, :])

        def v_chunk(t):
            ts = slice(128 * t, 128 * t + 128)
            pv = pmm.tile([128, 512], F32, name="pmm", bufs=2)
            for kc in range(4):
                nc.tensor.matmul(pv, vtin[:, kc, ts], wv_sb[:, kc, :],
                                 start=(kc == 0), stop=(kc == 3))
            with nc.allow_low_precision("fp16 activations"):
                nc.vector.tensor_copy(
                    vaug[:, t, :, 0:64],
                    pv.rearrange("p (h v) -> p h v", h=8))

        # ---- Prefix: all LN stats (so ACT never swaps tables mid-exp),
        # K proj jc=0/1, Q proj jc=0/1 n2=0, V proj t=0..3. Interleaved by
        # DMA arrival order. ----
        for n2 in range(4):
            ln_stats(ktin, n2, auxk, rstdK, pre)
            proj_chunk(kTs, ktin, wk_sb, ak_sb, auxk, rstdK, 0, n2)
            proj_chunk(kTs, ktin, wk_sb, ak_sb, auxk, rstdK, 1, n2)
            if n2 < 2:
                ln_stats(qtin, n2, auxq, rstdQ, pre)
        proj_chunk(qTs, qtin, wq_sb, aq_sb, auxq, rstdQ, 0, 0)
        proj_chunk(qTs, qtin, wq_sb, aq_sb, auxq, rstdQ, 1, 0)
        pre_ctx.__exit__(None, None, None)  # release stats banks for psL

        # Streaming tasks: remaining projections and the c=0 finale run
        # through the attention window (ACT exp stream is the pacing item
        # there once fp8 logits land; PE slack absorbs these).
        tasks = {}

        def add_task(u, fn):
            tasks.setdefault(u, []).append(fn)

        def mk_proj(dst, xin, w_sb, a_sb, aux, rstd_store, jc, n2):
            return lambda: proj_chunk(dst, xin, w_sb, a_sb, aux, rstd_store,
                                      jc, n2)

        for t in range(16):
            add_task(t, lambda t=t: v_chunk(t))
        for n2 in range(4):
            add_task(17 + 2 * n2, mk_proj(kTs, ktin, wk_sb, ak_sb,
                                          auxk, rstdK, 2, n2))
            add_task(27 + 2 * n2, mk_proj(kTs, ktin, wk_sb, ak_sb,
                                          auxk, rstdK, 3, n2))
        add_task(25, mk_proj(qTs, qtin, wq_sb, aq_sb, auxq, rstdQ, 2, 0))
        add_task(35, mk_proj(qTs, qtin, wq_sb, aq_sb, auxq, rstdQ, 3, 0))
        for jc in range(4):
            add_task(49 + 2 * jc, mk_proj(qTs, qtin, wq_sb, aq_sb,
                                          auxq, rstdQ, jc, 1))

        # ---- Attention: units (c, P, p); c-major so the c=0 finale can
        # stream inside the window. ----
        at_psL = top.enter_context(tc.tile_pool(name="at_psL", bufs=1,
                                                space="PSUM"))
        at_po = top.enter_context(tc.tile_pool(name="at_po", bufs=1,
                                               space="PSUM"))
        at_sb = top.enter_context(tc.tile_pool(name="at_sb", bufs=1))

        units = [(c, P, p) for c in range(2) for P in range(4)
                 for p in range(16)]
        LAG = 4
        pend = {}   # unit idx -> (P, c, p, psO pair, ex tile)

        def emit_O(u):
            P, c, p, ps_o, ex = pend.pop(u)
            for hh in range(2):
                nc.tensor.matmul(ps_o[hh], vaug[:, p, 2 * P + hh, :],
                                 ex[:, 512 * hh: 512 * hh + 512],
                                 start=(p == 0), stop=(p == 15))
            if p == 15:
                cs = slice(512 * c, 512 * c + 512)
                for hh in range(2):
                    # Evacuate the accumulator bank with two quick aligned
                    # DVE copies so the next (P,c) group's first O-matmul
                    # isn't blocked behind the whole normalize chain.
                    den = at_sb.tile([65, 512], F32, name="rr", bufs=2)
                    nc.vector.tensor_copy(den[64:65, :], ps_o[hh][64:65, :])
                    nst = at_sb.tile([64, 512], F32, name="nst", bufs=2)
                    nc.vector.tensor_copy(nst, ps_o[hh][0:64, :])
                    # recip_approx only works from SBUF at partition 0: DMA
                    # the den row down, then reciprocal + broadcast.
                    den0 = at_sb.tile([1, 512], F32, name="tl32", bufs=6)
                    nc.gpsimd.dma_start(out=den0, in_=den[64:65, :])
                    rr0 = at_sb.tile([1, 512], F32, name="tl32", bufs=6)
                    nc.vector.reciprocal_approx_fast(rr0, den0)
                    rrb = at_sb.tile([64, 512], F32, name="tl32", bufs=6)
                    nc.gpsimd.partition_broadcast(rrb, rr0)
                    rb = 64 * hh
                    if hh == 0:
                        # rows align with oTs: write the normalized block
                        # in place, no DMA hop.
                        with nc.allow_low_precision("f32r storage"):
                            nc.vector.tensor_mul(oTs[0:64, P, cs], nst, rrb)
                    else:
                        ost = at_sb.tile([64, 512], F32, name="tl32", bufs=6)
                        nc.vector.tensor_mul(ost, nst, rrb)
                        nc.gpsimd.dma_start(
                            out=oTs[rb:rb + 64, P, cs].bitcast(F32), in_=ost)

        # ---- Finale: LN fold (prescale form) + Wo + gelu + residual.
        # One block per 512-token chunk; block n2=0 runs as a single task
        # inside the window (its ACT ops sit consecutively in the ACT
        # stream: one Sqrt+Gelu table excursion). ----
        def fin_block(n2):
            ns = slice(512 * n2, 512 * n2 + 512)
            ps_sum = pmm.tile([128, 512], F32, name="pmm", bufs=2)
            for jc in range(4):
                nc.tensor.matmul(ps_sum, onesr, oTs[:, jc, ns],
                                 start=(jc == 0), stop=(jc == 3))
            ps_ssq = pmm.tile([128, 512], F32, name="pmm", bufs=2)
            for jc in range(4):
                sqo = work.tile([128, 512], F32R, name="w32", bufs=6)
                with nc.allow_low_precision("f32r keeps fp32 storage"):
                    nc.vector.tensor_mul(sqo, oTs[:, jc, ns], oTs[:, jc, ns])
                nc.tensor.matmul(ps_ssq, onesr, sqo,
                                 start=(jc == 0), stop=(jc == 3))
            m_b = work.tile([128, 512], F32, name="w32", bufs=6)
            nc.scalar.mul(m_b, ps_sum, 1.0 / 512.0)
            t2 = work.tile([128, 512], F32, name="w32", bufs=6)
            nc.vector.tensor_mul(t2, m_b, ps_sum)
            dv = work.tile([128, 512], F32, name="w32", bufs=6)
            nc.vector.tensor_sub(dv, ps_ssq, t2)
            std_b = work.tile([128, 512], F32, name="w32", bufs=6)
            nc.scalar.activation(std_b, dv, AF.Sqrt, bias=eps_t,
                                 scale=1.0 / 512.0)
            r_b = work.tile([128, 512], F32, name="w32", bufs=6)
            nc.vector.reciprocal_approx_fast(r_b, std_b)
            with nc.allow_low_precision("f32r keeps fp32 storage"):
                nc.vector.tensor_mul(auxo[0:1, ns], m_b[0:1, :], r_b[0:1, :])
                for jc in range(4):
                    nc.vector.tensor_mul(oTs[:, jc, ns], oTs[:, jc, ns], r_b)
            for jc in range(4):
                js = slice(128 * jc, 128 * jc + 128)
                pg = pmm.tile([128, 512], F32, name="pmm", bufs=2)
                for kc in range(4):
                    nc.tensor.matmul(pg, wo_sb[:, kc, js], oTs[:, kc, ns],
                                     start=(kc == 0), stop=False)
                nc.tensor.matmul(pg, ao_sb[:, js], auxo[:, ns],
                                 start=False, stop=True)
                pbm = pmm.tile([128, 512], F32, name="pmm", bufs=2)
                nc.tensor.matmul(pbm, grow_sb[0:1, js], auxo[0:1, ns],
                                 start=True, stop=True)
                gl = work.tile([128, 512], F32, name="w32", bufs=6)
                nc.scalar.activation(gl, pg, AF.Gelu)
                u2 = work.tile([128, 512], F32, name="w32", bufs=6)
                # u2 = oTs_scaled*g - m*r*g   (oTs already prescaled by r)
                nc.vector.scalar_tensor_tensor(
                    u2, oTs[:, jc, ns], gb_sb[:, 0, jc:jc + 1], pbm,
                    op0=ALU.mult, op1=ALU.subtract)
                of = work.tile([128, 512], F16, name="wof", bufs=4)
                with nc.allow_low_precision("fp16 output"):
                    nc.vector.scalar_tensor_tensor(
                        of, u2, gb_sb[:, 1, jc:jc + 1], gl,
                        op0=ALU.add, op1=ALU.add)
                nc.gpsimd.dma_start(out=out_d[:, jc, ns], in_=of)

        add_task(68, lambda: fin_block(0))

        ps_o_cur = None
        for u, (c, P, p) in enumerate(units):
            if p == 0:
                ps_o_cur = [at_po.tile([65, 512], F32, name=f"po{hh}",
                                       bufs=1) for hh in range(2)]
            psL = at_psL.tile([128, 1024], F32, name="psL", bufs=2)
            ks = slice(128 * p, 128 * p + 128)
            cs = slice(512 * c, 512 * c + 512)
            for hh in range(2):
                rb = 64 * hh
                nc.tensor.matmul(psL[:, 512 * hh: 512 * hh + 512],
                                 kTs[rb:rb + 64, P, ks],
                                 qTs[rb:rb + 64, P, cs],
                                 start=True, stop=True)
            ex = at_sb.tile([128, 1024], F16, name="ex", bufs=LAG + 1)
            nc.scalar.activation(ex, psL, AF.Exp, scale=1.0 / TEMP)
            pend[u] = (P, c, p, ps_o_cur, ex)
            if u >= LAG:
                emit_O(u - LAG)
            for fn in tasks.pop(u, ()):
                fn()
        for u in range(len(units) - LAG, len(units)):
            emit_O(u)

        fin_block(1)

    nc.compile()
    return nc


def _chunk_fm(x):
    """[512, N] feature-major -> [128, 4, N] (partition, chunk, col)."""
    n = x.shape[1]
    return np.ascontiguousarray(x.reshape(4, 128, n).transpose(1, 0, 2))


def _prep_inputs(Q, K, V, Wq, Wk, Wv, Wo, g, b, go, bo):
    WqT = np.ascontiguousarray((Wq * g[None, :]).T)
    WkT = np.ascontiguousarray((Wk * g[None, :]).T)
    WvT = np.ascontiguousarray(Wv.T)
    WoT = np.ascontiguousarray((Wo * go[None, :]).T)
    f16 = np.float16
    shared = {
        f"wq_{SALT}": _chunk_fm(WqT).astype(f16),
        f"wk_{SALT}": _chunk_fm(WkT).astype(f16),
        f"wv_{SALT}": _chunk_fm(WvT).astype(f16),
        f"wo_{SALT}": _chunk_fm(WoT),
        f"aq_{SALT}": np.ascontiguousarray(np.stack([-WqT.sum(0), Wq @ b])),
        f"ak_{SALT}": np.ascontiguousarray(np.stack([-WkT.sum(0), Wk @ b])),
        f"ao_{SALT}": np.ascontiguousarray(np.stack([-WoT.sum(0), Wo @ bo])),
        f"gb_{SALT}": np.ascontiguousarray(
            np.stack([go.reshape(4, 128).T, bo.reshape(4, 128).T], axis=1)),
        f"grow_{SALT}": np.ascontiguousarray(go[None, :]),
    }
    in_maps = []
    for core in range(N_CORES):
        bi, half = core // 2, core % 2
        qs = slice(half * NQS, (half + 1) * NQS)
        m = dict(shared)
        m[f"salt_{SALT}"] = np.full((1, 8 + SALT_N), LN_EPS, np.float32)
        m[f"qt_{SALT}"] = _chunk_fm(np.ascontiguousarray(Q[bi, qs, :].T)).astype(f16)
        m[f"kt_{SALT}"] = _chunk_fm(np.ascontiguousarray(K[bi].T)).astype(f16)
        m[f"vt_{SALT}"] = _chunk_fm(np.ascontiguousarray(V[bi].T)).astype(f16)
        in_maps.append(m)
    return in_maps


def kernel(Q, K, V, Wq, Wk, Wv, Wo, ln_qk_g, ln_qk_b, ln_o_g, ln_o_b,
           _trace=False):
    args = [np.asarray(a, dtype=np.float32) for a in
            (Q, K, V, Wq, Wk, Wv, Wo, ln_qk_g, ln_qk_b, ln_o_g, ln_o_b)]
    if "nc" not in _CACHE:
        _CACHE["nc"] = _build_program()
    nc = _CACHE["nc"]
    in_maps = _prep_inputs(*args)
    res = run_bass_kernel_spmd(nc, in_maps, core_ids=list(range(N_CORES)),
                               trace=_trace)
    _CACHE["last_results"] = res
    out = np.empty((B, NQ, D), dtype=np.float32)
    for core in range(N_CORES):
        bi, half = core // 2, core % 2
        o = res.results[core][f"out_{SALT}"].astype(np.float32)  # [128,4,NQS]
        out[bi, half * NQS : (half + 1) * NQS, :] = (
            o.transpose(1, 0, 2).reshape(D, NQS).T)
    return out


# revision 10
# speedup vs baseline: 1.2227x; 1.2227x over previous
# Cross-attention kernel for Trainium2, 8 NeuronCores.
#
# Sharding: data-parallel over (batch, query-half): core = 2*b + half handles
# batch b, queries [half*1024, (half+1)*1024). No collectives.
#
# On-device layout is feature-major: activations live as [feature, token] in
# fp16. Both layernorms fold into the projections via the postscale form
#   LN(x) @ W'.T = (x @ W' + [-S; bq] x [m; std]) * rstd,
# so the PSUM->SBUF evacuation copy becomes the rstd multiply. The finale
# keeps the prescale form so Gelu reads its PSUM accumulator directly.
#
# v15 structure: the attention unit order is c-major (query-chunk outer
# across head-pairs) so the c=0 finale streams inside the exp window; V and
# the later K/Q projection chunks stream through the window as PE tasks under
# the ACT exp stream. Input DMAs ride four independent engine queues (sync:
# K, vector: V, scalar: weights+Q, gpsimd: small internal rows) so the
# critical K chunk + weights land in ~2us instead of queueing behind 5MB.
# All LN stats run in the prefix so the ACT engine never reloads the Exp
# table mid-stream; the two finale blocks each cost one Sqrt+Gelu table
# excursion. Output is written fp16 (absmax ~5, quantization ~3e-4 abs).
import os
import sys
import tempfile

os.environ["NEURON_COMPILE_CACHE_URL"] = tempfile.mkdtemp(prefix="neff_cache_")
os.environ["AXON_CASSETTE_SALT"] = f"ca-{os.getpid()}-{os.urandom(4).hex()}"

for _p in ("/opt/trn_rl_repo",):
    if os.path.isdir(_p) and _p not in sys.path:
        sys.path.insert(0, _p)

import numpy as np
from contextlib import ExitStack

import concourse.bass as bass
import concourse.tile as tile
from concourse import bacc, mybir
from concourse.bass_utils import run_bass_kernel_spmd

F32 = mybir.dt.float32
F32R = mybir.dt.float32r
F16 = mybir.dt.float16
AF = mybir.ActivationFunctionType
ALU = mybir.AluOpType

B, NQ, NK, D = 4, 2048, 2048, 512
H, DH = 8, 64
NQS = NQ // 2  # queries per core
TEMP = float(np.sqrt(512.0))
LN_EPS = 1e-5
N_CORES = 8

_CACHE = {}
SALT = "v17"
SALT_N = 17


def _build_program():
    nc = bacc.Bacc("TRN2", target_bir_lowering=False, debug=False)

    def din(name, shape, dt=F32):
        return nc.dram_tensor(f"{name}_{SALT}", shape, dt,
                              kind="ExternalInput").ap()

    qt_d = din("qt", [128, 4, NQS], F16)
    kt_d = din("kt", [128, 4, NK], F16)
    vt_d = din("vt", [128, 4, NK], F16)
    wq_d = din("wq", [128, 4, D], F16)
    wk_d = din("wk", [128, 4, D], F16)
    wv_d = din("wv", [128, 4, D], F16)
    wo_d = din("wo", [128, 4, D], F32R)
    aq_d = din("aq", [2, D], F32R)
    ak_d = din("ak", [2, D], F32R)
    ao_d = din("ao", [2, D], F32R)
    gb_d = din("gb", [128, 2, 4], F32)
    grow_d = din("grow", [1, D], F32R)
    salt_d = din("salt", [1, 8 + SALT_N], F32)
    out_d = nc.dram_tensor(f"out_{SALT}", [128, 4, NQS], F16,
                           kind="ExternalOutput").ap()

    with tile.TileContext(nc) as tc, ExitStack() as top:
        persist = top.enter_context(tc.tile_pool(name="persist", bufs=1))
        qTs = persist.tile([128, 4, NQS], F16)        # projected q
        kTs = persist.tile([128, 4, NK], F16)         # projected k
        vaug = persist.tile([128, 16, 8, 65], F16)    # v natural + ones col
        oTs = persist.tile([128, 4, NQS], F32R)       # attention out
        qtin = persist.tile([128, 4, NQS], F16)
        ktin = persist.tile([128, 4, NK], F16)
        vtin = persist.tile([128, 4, NK], F16)
        wq_sb = persist.tile([128, 4, D], F16)
        wk_sb = persist.tile([128, 4, D], F16)
        wv_sb = persist.tile([128, 4, D], F16)
        wo_sb = persist.tile([128, 4, D], F32R)
        aq_sb = persist.tile([2, D], F32R)
        ak_sb = persist.tile([2, D], F32R)
        ao_sb = persist.tile([2, D], F32R)
        gb_sb = persist.tile([128, 2, 4], F32)
        grow_sb = persist.tile([1, D], F32R)
        auxq = persist.tile([2, NQS], F32R)           # [m; std] rows for Q
        auxk = persist.tile([2, NK], F32R)
        auxo = persist.tile([2, NQS], F32R)           # [m*r; 1] rows, finale
        rstdQ = persist.tile([128, 2, 512], F32)      # replicated 1/std per
        rstdK = persist.tile([128, 4, 512], F32)      # 512-token chunk
        onesh = persist.tile([128, 128], F16)
        onesr = persist.tile([128, 128], F32R)
        eps_t = persist.tile([128, 1], F32)

        # Three independent DMA queues (only SP/ACT/gpsimd may trigger):
        # sync carries the bulk (K, V, finale weights); scalar triggers ONLY
        # the critical small loads then frees the ACT engine for the stats
        # chain (each 5th+ trigger on a queue waits on semaphore recycling,
        # so a long trigger list pins its engine for the whole load);
        # gpsimd stays near-empty for latency-critical rows issued
        # mid-stream.
        nc.scalar.dma_start(out=eps_t,
                            in_=salt_d[0:1, 0:1].to_broadcast([128, 1]))
        nc.scalar.dma_start(out=wk_sb, in_=wk_d)
        nc.scalar.dma_start(out=wq_sb, in_=wq_d)
        for n2 in range(4):
            ns = slice(512 * n2, 512 * n2 + 512)
            nc.sync.dma_start(out=ktin[:, :, ns], in_=kt_d[:, :, ns])
        for n2 in range(2):
            ns = slice(512 * n2, 512 * n2 + 512)
            nc.scalar.dma_start(out=qtin[:, :, ns], in_=qt_d[:, :, ns])
        nc.gpsimd.dma_start(out=aq_sb, in_=aq_d)
        nc.gpsimd.dma_start(out=ak_sb, in_=ak_d)
        nc.scalar.dma_start(out=wv_sb, in_=wv_d)
        for n2 in range(4):
            ns = slice(512 * n2, 512 * n2 + 512)
            nc.sync.dma_start(out=vtin[:, :, ns], in_=vt_d[:, :, ns])
        nc.sync.dma_start(out=gb_sb, in_=gb_d)
        nc.sync.dma_start(out=grow_sb, in_=grow_d)
        nc.sync.dma_start(out=ao_sb, in_=ao_d)
        nc.sync.dma_start(out=wo_sb, in_=wo_d)
        nc.vector.memset(onesh, 1.0)
        # memset cannot target f32r; write through an f32 view (same bytes)
        nc.vector.memset(onesr.bitcast(F32), 1.0)
        nc.vector.memset(vaug[:, :, :, 64], 1.0)
        # row 0 is overwritten with m*r in the finale; row 1 stays all-ones
        nc.vector.memset(auxo.bitcast(F32), 1.0)

        pmm = top.enter_context(tc.tile_pool(name="pmm", bufs=1, space="PSUM"))
        work = top.enter_context(tc.tile_pool(name="work", bufs=1))
        # Prefix-only stats accumulators, released before the attention
        # pools open.
        pre_ctx = tc.tile_pool(name="pre", bufs=1, space="PSUM")
        pre = pre_ctx.__enter__()

        def ln_stats(xin, n2, aux, rstd_store, pool):
            """Column LN stats of token chunk n2 of xin [128, 4, *]; fills
            aux rows [m; std] and rstd_store[:, n2, :] (replicated)."""
            ns = slice(512 * n2, 512 * n2 + 512)
            ps_sum = pool.tile([128, 512], F32, name="st", bufs=4)
            for kc in range(4):
                nc.tensor.matmul(ps_sum, onesh, xin[:, kc, ns],
                                 start=(kc == 0), stop=(kc == 3))
            ps_ssq = pool.tile([128, 512], F32, name="st", bufs=4)
            for kc in range(4):
                sq = work.tile([128, 512], F16, name="sq", bufs=3)
                with nc.allow_low_precision("squares in fp16"):
                    nc.vector.tensor_mul(sq, xin[:, kc, ns], xin[:, kc, ns])
                nc.tensor.matmul(ps_ssq, onesh, sq,
                                 start=(kc == 0), stop=(kc == 3))
            m_b = work.tile([128, 512], F32, name="w32", bufs=6)
            nc.scalar.mul(m_b, ps_sum, 1.0 / 512.0)
            t2 = work.tile([128, 512], F32, name="w32", bufs=6)
            nc.vector.tensor_mul(t2, m_b, ps_sum)
            dv = work.tile([128, 512], F32, name="w32", bufs=6)
            nc.vector.tensor_sub(dv, ps_ssq, t2)
            std_b = work.tile([128, 512], F32, name="w32", bufs=6)
            nc.scalar.activation(std_b, dv, AF.Sqrt, bias=eps_t,
                                 scale=1.0 / 512.0)
            nc.vector.reciprocal_approx_fast(rstd_store[:, n2, :], std_b)
            # engine ops cannot move data across partitions; DMA the stat
            # rows into the K=2 aux operand instead (gpsimd queue: empty,
            # 25ns trigger).
            nc.gpsimd.dma_start(out=aux[0:1, ns].bitcast(F32), in_=m_b[0:1, :])
            nc.gpsimd.dma_start(out=aux[1:2, ns].bitcast(F32),
                                in_=std_b[0:1, :])

        def proj_chunk(dst, xin, w_sb, a_sb, aux, rstd_store, jc, n2):
            """dst[:, jc, ns] = (sum_kc w'[:,kc,js].T @ x[:,kc,ns] + aux)*r."""
            ns = slice(512 * n2, 512 * n2 + 512)
            js = slice(128 * jc, 128 * jc + 128)
            pg = pmm.tile([128, 512], F32, name="pmm", bufs=2)
            for kc in range(4):
                nc.tensor.matmul(pg, w_sb[:, kc, js], xin[:, kc, ns],
                                 start=(kc == 0), stop=False)
            nc.tensor.matmul(pg, a_sb[:, js], aux[:, ns],
                             start=False, stop=True)
            with nc.allow_low_precision("fp16 activations"):
                nc.vector.tensor_mul(dst[:, jc, ns], pg, rstd_store[:, n2, :])

        def v_chunk(t):
            ts = slice(128 * t, 128 * t + 128)
            pv = pmm.tile([128, 512], F32, name="pmm", bufs=2)
            for kc in range(4):
                nc.tensor.matmul(pv, vtin[:, kc, ts], wv_sb[:, kc, :],
                                 start=(kc == 0), stop=(kc == 3))
            with nc.allow_low_precision("fp16 activations"):
                nc.vector.tensor_copy(
                    vaug[:, t, :, 0:64],
                    pv.rearrange("p (h v) -> p h v", h=8))

        # ---- Prefix: all LN stats first (so ACT never swaps tables
        # mid-exp and no stats chunk waits on an aux DMA round-trip), in
        # DMA arrival order, then just the two projections gating unit 0.
        # Everything else streams through the window. ----
        ln_stats(ktin, 0, auxk, rstdK, pre)
        ln_stats(qtin, 0, auxq, rstdQ, pre)
        ln_stats(ktin, 1, auxk, rstdK, pre)
        ln_stats(qtin, 1, auxq, rstdQ, pre)
        ln_stats(ktin, 2, auxk, rstdK, pre)
        ln_stats(ktin, 3, auxk, rstdK, pre)
        proj_chunk(kTs, ktin, wk_sb, ak_sb, auxk, rstdK, 0, 0)
        proj_chunk(qTs, qtin, wq_sb, aq_sb, auxq, rstdQ, 0, 0)
        pre_ctx.__exit__(None, None, None)  # release stats banks for psL

        # Streaming tasks: remaining projections and the c=0 finale run
        # through the attention window (ACT exp stream is the pacing item
        # there once fp8 logits land; PE slack absorbs these).
        tasks = {}

        def add_task(u, fn):
            tasks.setdefault(u, []).append(fn)

        def mk_proj(dst, xin, w_sb, a_sb, aux, rstd_store, jc, n2):
            return lambda: proj_chunk(dst, xin, w_sb, a_sb, aux, rstd_store,
                                      jc, n2)

        for t in range(16):
            add_task(t, lambda t=t: v_chunk(t))
        # K proj (jc=P, n2) due by unit 16*P + 4*n2 - 1; Q proj (jc, n2=c)
        # due by unit 64*c + 16*jc - 1. V chunk t due by unit t + LAG - 1.
        kslot = {(0, 1): 0, (0, 2): 4, (0, 3): 8,
                 (1, 0): 12, (1, 1): 14, (1, 2): 18, (1, 3): 22,
                 (2, 0): 26, (2, 1): 30, (2, 2): 34, (2, 3): 38,
                 (3, 0): 42, (3, 1): 46, (3, 2): 50, (3, 3): 54}
        for (jc, n2), u in kslot.items():
            add_task(u, mk_proj(kTs, ktin, wk_sb, ak_sb, auxk, rstdK, jc, n2))
        qslot = {(1, 0): 10, (2, 0): 24, (3, 0): 40,
                 (0, 1): 58, (1, 1): 60, (2, 1): 62, (3, 1): 64}
        for (jc, n2), u in qslot.items():
            add_task(u, mk_proj(qTs, qtin, wq_sb, aq_sb, auxq, rstdQ, jc, n2))

        # ---- Attention: units (c, P, p); c-major so the c=0 finale can
        # stream inside the window. ----
        at_psL = top.enter_context(tc.tile_pool(name="at_psL", bufs=1,
                                                space="PSUM"))
        at_po = top.enter_context(tc.tile_pool(name="at_po", bufs=1,
                                               space="PSUM"))
        at_sb = top.enter_context(tc.tile_pool(name="at_sb", bufs=1))

        units = [(c, P, p) for c in range(2) for P in range(4)
                 for p in range(16)]
        LAG = 4
        pend = {}   # unit idx -> (P, c, p, psO pair, ex tile)

        def emit_O(u):
            P, c, p, ps_o, ex = pend.pop(u)
            for hh in range(2):
                nc.tensor.matmul(ps_o[hh], vaug[:, p, 2 * P + hh, :],
                                 ex[:, 512 * hh: 512 * hh + 512],
                                 start=(p == 0), stop=(p == 15))
            if p == 15:
                cs = slice(512 * c, 512 * c + 512)
                for hh in range(2):
                    # Evacuate the accumulator bank with two quick aligned
                    # DVE copies so the next (P,c) group's first O-matmul
                    # isn't blocked behind the whole normalize chain.
                    den = at_sb.tile([65, 512], F32, name="rr", bufs=2)
                    nc.vector.tensor_copy(den[64:65, :], ps_o[hh][64:65, :])
                    nst = at_sb.tile([64, 512], F32, name="nst", bufs=2)
                    nc.vector.tensor_copy(nst, ps_o[hh][0:64, :])
                    # recip_approx only works from SBUF at partition 0: DMA
                    # the den row down, then reciprocal + broadcast.
                    den0 = at_sb.tile([1, 512], F32, name="tl32", bufs=6)
                    nc.gpsimd.dma_start(out=den0, in_=den[64:65, :])
                    rr0 = at_sb.tile([1, 512], F32, name="tl32", bufs=6)
                    nc.vector.reciprocal_approx_fast(rr0, den0)
                    rrb = at_sb.tile([64, 512], F32, name="tl32", bufs=6)
                    nc.gpsimd.partition_broadcast(rrb, rr0)
                    rb = 64 * hh
                    if hh == 0:
                        # rows align with oTs: write the normalized block
                        # in place, no DMA hop.
                        with nc.allow_low_precision("f32r storage"):
                            nc.vector.tensor_mul(oTs[0:64, P, cs], nst, rrb)
                    else:
                        ost = at_sb.tile([64, 512], F32, name="tl32", bufs=6)
                        nc.vector.tensor_mul(ost, nst, rrb)
                        nc.gpsimd.dma_start(
                            out=oTs[rb:rb + 64, P, cs].bitcast(F32), in_=ost)

        # ---- Finale: LN fold (prescale form) + Wo + gelu + residual.
        # One block per 512-token chunk; block n2=0 runs as a single task
        # inside the window (its ACT ops sit consecutively in the ACT
        # stream: one Sqrt+Gelu table excursion). ----
        def fin_block(n2):
            ns = slice(512 * n2, 512 * n2 + 512)
            ps_sum = pmm.tile([128, 512], F32, name="pmm", bufs=2)
            for jc in range(4):
                nc.tensor.matmul(ps_sum, onesr, oTs[:, jc, ns],
                                 start=(jc == 0), stop=(jc == 3))
            ps_ssq = pmm.tile([128, 512], F32, name="pmm", bufs=2)
            for jc in range(4):
                sqo = work.tile([128, 512], F32R, name="w32", bufs=6)
                with nc.allow_low_precision("f32r keeps fp32 storage"):
                    nc.vector.tensor_mul(sqo, oTs[:, jc, ns], oTs[:, jc, ns])
                nc.tensor.matmul(ps_ssq, onesr, sqo,
                                 start=(jc == 0), stop=(jc == 3))
            m_b = work.tile([128, 512], F32, name="w32", bufs=6)
            nc.scalar.mul(m_b, ps_sum, 1.0 / 512.0)
            t2 = work.tile([128, 512], F32, name="w32", bufs=6)
            nc.vector.tensor_mul(t2, m_b, ps_sum)
            dv = work.tile([128, 512], F32, name="w32", bufs=6)
            nc.vector.tensor_sub(dv, ps_ssq, t2)
            std_b = work.tile([128, 512], F32, name="w32", bufs=6)
            nc.scalar.activation(std_b, dv, AF.Sqrt, bias=eps_t,
                                 scale=1.0 / 512.0)
            r_b = work.tile([128, 512], F32, name="w32", bufs=6)
            nc.vector.reciprocal_approx_fast(r_b, std_b)
            with nc.allow_low_precision("f32r keeps fp32 storage"):
                nc.vector.tensor_mul(auxo[0:1, ns], m_b[0:1, :], r_b[0:1, :])
                for jc in range(4):
                    nc.vector.tensor_mul(oTs[:, jc, ns], oTs[:, jc, ns], r_b)
            for jc in range(4):
                js = slice(128 * jc, 128 * jc + 128)
                pg = pmm.tile([128, 512], F32, name="pmm", bufs=2)
                for kc in range(4):
                    nc.tensor.matmul(pg, wo_sb[:, kc, js], oTs[:, kc, ns],
                                     start=(kc == 0), stop=False)
                nc.tensor.matmul(pg, ao_sb[:, js], auxo[:, ns],
                                 start=False, stop=True)
                pbm = pmm.tile([128, 512], F32, name="pmm", bufs=2)
                nc.tensor.matmul(pbm, grow_sb[0:1, js], auxo[0:1, ns],
                                 start=True, stop=True)
                gl = work.tile([128, 512], F32, name="w32", bufs=6)
                nc.scalar.activation(gl, pg, AF.Gelu)
                u2 = work.tile([128, 512], F32, name="w32", bufs=6)
                # u2 = oTs_scaled*g - m*r*g   (oTs already prescaled by r)
                nc.vector.scalar_tensor_tensor(
                    u2, oTs[:, jc, ns], gb_sb[:, 0, jc:jc + 1], pbm,
                    op0=ALU.mult, op1=ALU.subtract)
                of = work.tile([128, 512], F16, name="wof", bufs=4)
                with nc.allow_low_precision("fp16 output"):
                    nc.vector.scalar_tensor_tensor(
                        of, u2, gb_sb[:, 1, jc:jc + 1], gl,
                        op0=ALU.add, op1=ALU.add)
                nc.gpsimd.dma_start(out=out_d[:, jc, ns], in_=of)

        add_task(68, lambda: fin_block(0))

        ps_o_cur = None
        for u, (c, P, p) in enumerate(units):
            if p == 0:
                ps_o_cur = [at_po.tile([65, 512], F32, name=f"po{hh}",
                                       bufs=1) for hh in range(2)]
            psL = at_psL.tile([128, 1024], F32, name="psL", bufs=2)
            ks = slice(128 * p, 128 * p + 128)
            cs = slice(512 * c, 512 * c + 512)
            for hh in range(2):
                rb = 64 * hh
                nc.tensor.matmul(psL[:, 512 * hh: 512 * hh + 512],
                                 kTs[rb:rb + 64, P, ks],
                                 qTs[rb:rb + 64, P, cs],
                                 start=True, stop=True)
            ex = at_sb.tile([128, 1024], F16, name="ex", bufs=LAG + 1)
            nc.scalar.activation(ex, psL, AF.Exp, scale=1.0 / TEMP)
            pend[u] = (P, c, p, ps_o_cur, ex)
            if u >= LAG:
                emit_O(u - LAG)
            for fn in tasks.pop(u, ()):
                fn()
        for u in range(len(units) - LAG, len(units)):
            emit_O(u)

        fin_block(1)

    nc.compile()
    return nc


def _chunk_fm(x):
    """[512, N] feature-major -> [128, 4, N] (partition, chunk, col)."""
    n = x.shape[1]
    return np.ascontiguousarray(x.reshape(4, 128, n).transpose(1, 0, 2))


def _prep_inputs(Q, K, V, Wq, Wk, Wv, Wo, g, b, go, bo):
    WqT = np.ascontiguousarray((Wq * g[None, :]).T)
    WkT = np.ascontiguousarray((Wk * g[None, :]).T)
    WvT = np.ascontiguousarray(Wv.T)
    WoT = np.ascontiguousarray((Wo * go[None, :]).T)
    f16 = np.float16
    shared = {
        f"wq_{SALT}": _chunk_fm(WqT).astype(f16),
        f"wk_{SALT}": _chunk_fm(WkT).astype(f16),
        f"wv_{SALT}": _chunk_fm(WvT).astype(f16),
        f"wo_{SALT}": _chunk_fm(WoT),
        f"aq_{SALT}": np.ascontiguousarray(np.stack([-WqT.sum(0), Wq @ b])),
        f"ak_{SALT}": np.ascontiguousarray(np.stack([-WkT.sum(0), Wk @ b])),
        f"ao_{SALT}": np.ascontiguousarray(np.stack([-WoT.sum(0), Wo @ bo])),
        f"gb_{SALT}": np.ascontiguousarray(
            np.stack([go.reshape(4, 128).T, bo.reshape(4, 128).T], axis=1)),
        f"grow_{SALT}": np.ascontiguousarray(go[None, :]),
    }
    in_maps = []
    for core in range(N_CORES):
        bi, half = core // 2, core % 2
        qs = slice(half * NQS, (half + 1) * NQS)
        m = dict(shared)
        m[f"salt_{SALT}"] = np.full((1, 8 + SALT_N), LN_EPS, np.float32)
        m[f"qt_{SALT}"] = _chunk_fm(np.ascontiguousarray(Q[bi, qs, :].T)).astype(f16)
        m[f"kt_{SALT}"] = _chunk_fm(np.ascontiguousarray(K[bi].T)).astype(f16)
        m[f"vt_{SALT}"] = _chunk_fm(np.ascontiguousarray(V[bi].T)).astype(f16)
        in_maps.append(m)
    return in_maps


def kernel(Q, K, V, Wq, Wk, Wv, Wo, ln_qk_g, ln_qk_b, ln_o_g, ln_o_b,
           _trace=False):
    args = [np.asarray(a, dtype=np.float32) for a in
            (Q, K, V, Wq, Wk, Wv, Wo, ln_qk_g, ln_qk_b, ln_o_g, ln_o_b)]
    if "nc" not in _CACHE:
        _CACHE["nc"] = _build_program()
    nc = _CACHE["nc"]
    in_maps = _prep_inputs(*args)
    res = run_bass_kernel_spmd(nc, in_maps, core_ids=list(range(N_CORES)),
                               trace=_trace)
    _CACHE["last_results"] = res
    out = np.empty((B, NQ, D), dtype=np.float32)
    for core in range(N_CORES):
        bi, half = core // 2, core % 2
        o = res.results[core][f"out_{SALT}"].astype(np.float32)  # [128,4,NQS]
        out[bi, half * NQS : (half + 1) * NQS, :] = (
            o.transpose(1, 0, 2).reshape(D, NQS).T)
    return out
